# revision 22
# baseline (speedup 1.0000x reference)
"""kNN hypergraph kernel for Trainium2 (8 NeuronCores, Bass/Tile).

Problem: x [16, 256, 768] f32, k=16.
  flat = x.reshape(4096, 768)
  d2[i,j] = |flat_i - flat_j|^2 ; idx = 16 nearest (incl self)
  hypergraph[i, idx[i,:]] = 1 ; out[b,s,t] = sum_b2 hg[b*256+s, b2*256+t]
Output: [16, 256, 256] f32.

Strategy (row-sharded across 8 cores, 512 rows each):
  - Rank rows by s[i,j] = 2<x_i,x_j> - |x_j|^2 (per-row constant sq_i
    dropped). The 16 NN are the 16 LARGEST s per row.
  - s = hi2 @ hi'        (fp16 x fp16, full PE rate, 6 MMs/block)
      + e4m3(2x) @ e5m2(lo')   } both cross terms as fp8 DoubleRow
      + e5m2(lo2) @ e4m3(x')   } matmuls: K=256/instr, ~1.44x rate
    where hi2 = fp16(2x), lo2 = 2x - hi2, hi = fp16(x), lo = x - hi,
    all into one PSUM accumulation group (natural product scales).
    Residual error ~1e-3 (vs top-16 decision gaps ~1e-1): 4 of 65536
    neighbor slots differ from exact, rel err 7.6e-3 < 2e-2.
  - The exact fp32 -sq row (host pre-broadcast to 128 partitions) is
    added by the DVE drain (tensor_add s_sb = psum + nsq chunk); a
    K=2 matmul alternative costs a full extra N-pass per block.
  - Phase A: chunk-outer over chunks 0-2 (each contiguous-DMA moving
    chunk reused by all 4 row-tiles as it lands, so PE starts ~11us).
    Phase B: row-outer (each row-tile runs chunks 3-7 then finalizes,
    overlapping the next row-tile's matmuls -> short tail).
  - Top-16 per row: DVE max8 top-8 per 512-block (>8 of the true
    top-16 in one block happens for 3 of 4096 rows - within budget),
    3-op combine over the 8x8 union -> sigma = 16th largest.
  - Mask (s >= sigma) as two 2048-wide is_ge passes + bf16 add, then
    log-tree fold over the 16 batches of 256 cols, all on DVE
    (GpSimd tensor ops measured ~4x slower; ACT cannot tensor+tensor).
"""

import os

import numpy as np

B, S, D = 16, 256, 768
N = B * S            # 4096 points
NCORES = 8
M = N // NCORES      # 512 rows per core
JT = 6               # K planes of 128 (768 features)
NT = 8               # moving chunks of 512 columns
RT = M // 128        # 4 row-tiles of 128 per core
NEG = -3.0e38        # sentinel: far below any real s value

_cache = {}


def _build():
    import concourse.mybir as mybir
    import concourse.tile as tile
    from concourse import bacc

    f32 = mybir.dt.float32
    f16 = mybir.dt.float16
    bf16 = mybir.dt.bfloat16
    f8e4 = mybir.dt.float8e4
    f8e5 = mybir.dt.float8e5
    DR = mybir.MatmulPerfMode.DoubleRow

    nc = bacc.Bacc("TRN2", target_bir_lowering=False, debug=False,
                   num_devices=NCORES)

    # moving side (identical on all cores), chunk-major so each chunk is
    # one contiguous [128, 3072] DMA
    m16_d = nc.dram_tensor("m16", [NT * 128, JT * 512], f16,
                           kind="ExternalInput")
    mlo_d = nc.dram_tensor("mlo", [NT * 128, JT * 512], f8e5,
                           kind="ExternalInput")
    mhi_d = nc.dram_tensor("mhi", [NT * 128, JT * 512], f8e4,
                           kind="ExternalInput")
    # -sq pre-broadcast to 128 partitions (fp32 exact): the drain adds it
    nsq_d = nc.dram_tensor("nsq", [128, N], f32, kind="ExternalInput")
    # stationary side (this core's 512 rows)
    st16_d = nc.dram_tensor("st16", [128, JT * 512], f16,
                            kind="ExternalInput")
    sta_d = nc.dram_tensor("sta", [128, JT * 512], f8e4,
                           kind="ExternalInput")
    stb_d = nc.dram_tensor("stb", [128, JT * 512], f8e5,
                           kind="ExternalInput")
    out_d = nc.dram_tensor("out", [M, S], f32, kind="ExternalOutput")

    with tile.TileContext(nc) as tc:
        with (
            tc.tile_pool(name="weights", bufs=1) as wpool,
            tc.tile_pool(name="s", bufs=1) as spool,
            tc.tile_pool(name="m8", bufs=1) as m8pool,
            tc.tile_pool(name="fin", bufs=2) as fpool,
            tc.tile_pool(name="mask", bufs=1) as mpool,
            tc.tile_pool(name="outp", bufs=2) as opool,
            tc.tile_pool(name="psum", bufs=8, space="PSUM") as psum,
        ):
            # critical path first: the fp16 stationary + first moving chunk,
            # both split so round-0 matmuls can start before the full tile
            # lands (plane j is consumed in emission order)
            st16 = wpool.tile([128, JT, 512], f16, tag="st16", name="st16")
            m16_sb = [wpool.tile([128, JT, 512], f16, tag=f"m16_{n}",
                                 name=f"m16_{n}") for n in range(NT)]
            # critical early DMAs issue in parallel across engine queues
            # (each DMA_DIRECT2D descriptor costs ~0.65us of queue time,
            # so a single queue serializes the fill by ~0.65us per tile)
            sta = wpool.tile([128, JT, 512], f8e4, tag="sta", name="sta")
            stb = wpool.tile([128, JT, 512], f8e5, tag="stb", name="stb")
            mlo_sb = [wpool.tile([128, JT, 512], f8e5, tag=f"mlo_{n}",
                                 name=f"mlo_{n}") for n in range(NT)]
            mhi_sb = [wpool.tile([128, JT, 512], f8e4, tag=f"mhi_{n}",
                                 name=f"mhi_{n}") for n in range(NT)]
            nsq_sb = [wpool.tile([128, 512], f32, tag=f"nsq_{n}",
                                 name=f"nsq_{n}") for n in range(NT)]
            nc.sync.dma_start(out=st16[:, 0:1, :], in_=st16_d[:, :512])
            nc.scalar.dma_start(out=m16_sb[0][:, 0:2, :],
                                in_=m16_d[0:128, :2 * 512])
            nc.gpsimd.dma_start(out=st16[:, 1:3, :],
                                in_=st16_d[:, 512:3 * 512])
            nc.gpsimd.dma_start(out=m16_sb[0][:, 2:4, :],
                                in_=m16_d[0:128, 2 * 512:4 * 512])
            nc.sync.dma_start(out=st16[:, 3:6, :], in_=st16_d[:, 3 * 512:])
            nc.scalar.dma_start(out=m16_sb[0][:, 4:6, :],
                                in_=m16_d[0:128, 4 * 512:])
            nc.scalar.dma_start(out=sta, in_=sta_d[:, :])
            nc.gpsimd.dma_start(out=stb, in_=stb_d[:, :])
            nc.scalar.dma_start(out=mlo_sb[0], in_=mlo_d[0:128, :])
            nc.gpsimd.dma_start(out=mhi_sb[0], in_=mhi_d[0:128, :])
            nc.sync.dma_start(out=nsq_sb[0], in_=nsq_d[:, 0:512])
            nc.scalar.dma_start(out=m16_sb[1], in_=m16_d[128:256, :])
            nc.gpsimd.dma_start(out=mlo_sb[1], in_=mlo_d[128:256, :])
            nc.scalar.dma_start(out=mhi_sb[1], in_=mhi_d[128:256, :])
            nc.sync.dma_start(out=nsq_sb[1], in_=nsq_d[:, 512:1024])
            for n in range(2, NT):
                nc.sync.dma_start(out=m16_sb[n],
                                  in_=m16_d[n * 128:(n + 1) * 128, :])
                nc.sync.dma_start(out=mlo_sb[n],
                                  in_=mlo_d[n * 128:(n + 1) * 128, :])
                nc.sync.dma_start(out=mhi_sb[n],
                                  in_=mhi_d[n * 128:(n + 1) * 128, :])
                nc.sync.dma_start(out=nsq_sb[n],
                                  in_=nsq_d[:, n * 512:(n + 1) * 512])

            s_sb = [spool.tile([128, N], f32, tag=f"s{rt}", name=f"s{rt}")
                    for rt in range(RT)]
            m8s = [m8pool.tile([128, 64], f32, tag=f"m8_{rt}",
                               name=f"m8_{rt}") for rt in range(RT)]

            def finalize(rt):
                # sigma = 16th largest of the union of the 8 block top-8s
                c8 = fpool.tile([128, 8], f32, tag="c8", name="c8")
                m8x = fpool.tile([128, 64], f32, tag="m8x", name="m8x")
                d8 = fpool.tile([128, 8], f32, tag="d8", name="d8")
                nc.vector.max(out=c8, in_=m8s[rt])
                nc.vector.match_replace(out=m8x, in_to_replace=c8,
                                        in_values=m8s[rt], imm_value=NEG)
                nc.vector.max(out=d8, in_=m8x)
                sigma = d8[:, 7:8]

                # neighbor mask (s >= sigma): two single-ALU-pass is_ge
                # halves + one bf16 add beat one fused two-op pass
                eng = nc.vector
                H = N // 2
                mask = mpool.tile([128, H], bf16, tag="maskA",
                                  name=f"maskA{rt}")
                maskB = mpool.tile([128, H], bf16, tag="maskB",
                                   name=f"maskB{rt}")
                eng.tensor_scalar(mask, s_sb[rt][:, :H], sigma, None,
                                  op0=mybir.AluOpType.is_ge)
                eng.tensor_scalar(maskB, s_sb[rt][:, H:], sigma, None,
                                  op0=mybir.AluOpType.is_ge)
                eng.tensor_add(mask, mask, maskB)
                w = H // 2
                while w > S:
                    eng.tensor_add(mask[:, :w], mask[:, :w],
                                   mask[:, w:2 * w])
                    w //= 2
                o = opool.tile([128, S], f32, tag=f"o{rt % 2}",
                               name=f"o{rt}")
                eng.tensor_add(o, mask[:, :S], mask[:, S:2 * S])
                nc.sync.dma_start(
                    out=out_d[rt * 128:(rt + 1) * 128, :], in_=o)

            def mm16(ps, n, rt):
                rsl = slice(rt * 128, (rt + 1) * 128)
                for j in range(JT):
                    nc.tensor.matmul(
                        ps, st16[:, j:j + 1, rsl],
                        m16_sb[n][:, j:j + 1, :],
                        start=(j == 0), stop=False)

            def mmdr(ps, n, rt):
                rsl = slice(rt * 128, (rt + 1) * 128)
                for c in range(JT // 2):
                    nc.tensor.matmul(
                        ps, sta[:, 2 * c:2 * c + 2, rsl],
                        mlo_sb[n][:, 2 * c:2 * c + 2, :],
                        start=False, stop=False, perf_mode=DR)
                for c in range(JT // 2):
                    nc.tensor.matmul(
                        ps, stb[:, 2 * c:2 * c + 2, rsl],
                        mhi_sb[n][:, 2 * c:2 * c + 2, :],
                        start=False, stop=(c == JT // 2 - 1), perf_mode=DR)

            def drain(ps, n, rt):
                # drain adds the exact fp32 -sq row; block top-8 follows
                nsl = slice(n * 512, (n + 1) * 512)
                nc.vector.tensor_add(s_sb[rt][:, nsl], ps, nsq_sb[n])
                nc.vector.max(out=m8s[rt][:, n * 8:(n + 1) * 8],
                              in_=s_sb[rt][:, nsl])

            def block(n, rt):
                ps = psum.tile([128, 512], f32, tag="ps", name="ps")
                mm16(ps, n, rt)
                mmdr(ps, n, rt)
                drain(ps, n, rt)

            # round 0: all fp16 MMs first (their operands land first),
            # fp8 DR MMs after - by then sta/stb/mlo0/mhi0 have arrived,
            # so the PE does not stall waiting on the fp8 tiles
            ps0 = [psum.tile([128, 512], f32, tag="ps", name=f"ps0_{rt}")
                   for rt in range(RT)]
            for rt in range(RT):
                mm16(ps0[rt], 0, rt)
            for rt in range(RT):
                mmdr(ps0[rt], 0, rt)
                drain(ps0[rt], 0, rt)

            # phase A: chunk-outer over the first chunks (each moving
            # chunk is reused by all 4 row-tiles right after it lands)
            NA = 3
            for n in range(1, NA):
                for rt in range(RT):
                    block(n, rt)
            # phase B: row-outer so each row-tile finishes all its chunks
            # early and its finalize overlaps the next row-tile's matmuls
            for rt in range(RT):
                for n in range(NA, NT):
                    block(n, rt)
                finalize(rt)

    nc.compile()
    return nc


def _prep_inputs(x):
    import ml_dtypes
    e4 = ml_dtypes.float8_e4m3
    e5 = ml_dtypes.float8_e5m2

    flat = np.asarray(x, dtype=np.float32).reshape(N, D)
    sq = (flat.astype(np.float64) ** 2).sum(1)

    hi = flat.astype(np.float16)                      # rhs fp16
    lo = flat - hi.astype(np.float32)                 # rhs residual
    hi2 = (2.0 * flat).astype(np.float16)             # lhs fp16
    lo2 = 2.0 * flat - hi2.astype(np.float32)         # lhs residual

    def planes(a):
        # [4096, 768] -> [128, 6, 4096]: plane j row p = feature j*128+p
        return np.ascontiguousarray(
            a.T.reshape(JT, 128, N).transpose(1, 0, 2))

    def chunks(a):
        # [128, 6, 4096] -> [1024, 3072]: chunk-major moving layout
        return np.ascontiguousarray(
            a.reshape(128, JT, NT, 512).transpose(2, 0, 1, 3)
            .reshape(NT * 128, JT * 512))

    m16 = chunks(planes(hi))                          # fp16
    mlo = chunks(planes(lo.astype(e5).astype(np.float32))).astype(e5)
    mhi = chunks(planes(flat.astype(e4).astype(np.float32))).astype(e4)
    nsq = np.ascontiguousarray(
        np.broadcast_to((-sq).astype(np.float32), (128, N)))

    st16_full = planes(hi2)                           # [128, 6, 4096] fp16
    sta_full = planes((2.0 * flat).astype(e4).astype(np.float32))
    stb_full = planes(lo2.astype(e5).astype(np.float32))

    def st_core(a, c, dt):
        return np.ascontiguousarray(
            a[:, :, c * M:(c + 1) * M].reshape(128, JT * 512)).astype(dt)

    return m16, mlo, mhi, nsq, st16_full, sta_full, stb_full, st_core


def kernel(x, k):
    assert int(k) == 16
    import ml_dtypes
    e4 = ml_dtypes.float8_e4m3
    e5 = ml_dtypes.float8_e5m2
    (m16, mlo, mhi, nsq,
     st16_full, sta_full, stb_full, st_core) = _prep_inputs(x)

    if "nc" not in _cache:
        _cache["nc"] = _build()
    nc = _cache["nc"]

    in_maps = [
        {"m16": m16, "mlo": mlo, "mhi": mhi, "nsq": nsq,
         "st16": st_core(st16_full, c, np.float16),
         "sta": st_core(sta_full, c, e4),
         "stb": st_core(stb_full, c, e5)}
        for c in range(NCORES)
    ]

    from concourse.bass_utils import run_bass_kernel_spmd
    trace = bool(os.environ.get("KNN_TRACE"))
    if trace:
        try:
            from antenv.axon_hooks import get_axon_ntff_profile_hook
            if get_axon_ntff_profile_hook() is None:
                trace = False
        except ImportError:
            trace = False
    res = run_bass_kernel_spmd(nc, in_maps, core_ids=list(range(NCORES)),
                               trace=trace)
    if trace and res.exec_time_ns is not None:
        print(f"HW exec time: {res.exec_time_ns} ns")
        _cache["exec_time_ns"] = res.exec_time_ns

    out = np.concatenate([r["out"] for r in res.results], axis=0)
    return out.reshape(B, S, S)


# revision 23
# speedup vs baseline: 1.0460x; 1.0460x over previous
"""kNN hypergraph kernel for Trainium2 (8 NeuronCores, Bass/Tile).

Problem: x [16, 256, 768] f32, k=16.
  flat = x.reshape(4096, 768)
  d2[i,j] = |flat_i - flat_j|^2 ; idx = 16 nearest (incl self)
  hypergraph[i, idx[i,:]] = 1 ; out[b,s,t] = sum_b2 hg[b*256+s, b2*256+t]
Output: [16, 256, 256] f32.

Strategy (row-sharded across 8 cores, 512 rows each):
  - Rank rows by s[i,j] = 2<x_i,x_j> - |x_j|^2 (per-row constant sq_i
    dropped). The 16 NN are the 16 LARGEST s per row.
  - s = hi2 @ hi'        (fp16 x fp16, full PE rate, 6 MMs/block)
      + e4m3(2x) @ e5m2(lo')   } both cross terms as fp8 DoubleRow
      + e5m2(lo2) @ e4m3(x')   } matmuls: K=256/instr, ~1.44x rate
    where hi2 = fp16(2x), lo2 = 2x - hi2, hi = fp16(x), lo = x - hi,
    all into one PSUM accumulation group (natural product scales).
    Residual error ~1e-3 (vs top-16 decision gaps ~1e-1): 4 of 65536
    neighbor slots differ from exact, rel err 7.6e-3 < 2e-2.
  - The exact fp32 -sq row (host pre-broadcast to 128 partitions) is
    added by the DVE drain (tensor_add s_sb = psum + nsq chunk); a
    K=2 matmul alternative costs a full extra N-pass per block.
  - Phase A: chunk-outer over chunks 0-2 (each contiguous-DMA moving
    chunk reused by all 4 row-tiles as it lands, so PE starts ~11us).
    Phase B: row-outer (each row-tile runs chunks 3-7 then finalizes,
    overlapping the next row-tile's matmuls -> short tail).
  - Top-16 per row: DVE max8 top-8 per 512-block (>8 of the true
    top-16 in one block happens for 3 of 4096 rows - within budget),
    3-op combine over the 8x8 union -> sigma = 16th largest.
  - Mask (s >= sigma) as two 2048-wide is_ge passes + bf16 add, then
    log-tree fold over the 16 batches of 256 cols, all on DVE
    (GpSimd tensor ops measured ~4x slower; ACT cannot tensor+tensor).
"""

import os

import numpy as np

B, S, D = 16, 256, 768
N = B * S            # 4096 points
NCORES = 8
M = N // NCORES      # 512 rows per core
JT = 6               # K planes of 128 (768 features)
NT = 8               # moving chunks of 512 columns
RT = M // 128        # 4 row-tiles of 128 per core
NEG = -3.0e38        # sentinel: far below any real s value

_cache = {}


def _build():
    import concourse.mybir as mybir
    import concourse.tile as tile
    from concourse import bacc

    f32 = mybir.dt.float32
    f16 = mybir.dt.float16
    bf16 = mybir.dt.bfloat16
    f8e4 = mybir.dt.float8e4
    f8e5 = mybir.dt.float8e5
    DR = mybir.MatmulPerfMode.DoubleRow

    nc = bacc.Bacc("TRN2", target_bir_lowering=False, debug=False,
                   num_devices=NCORES)

    # moving side (identical on all cores), chunk-major so each chunk is
    # one contiguous [128, 3072] DMA
    m16_d = nc.dram_tensor("m16", [NT * 128, JT * 512], f16,
                           kind="ExternalInput")
    mlo_d = nc.dram_tensor("mlo", [NT * 128, JT * 512], f8e5,
                           kind="ExternalInput")
    mhi_d = nc.dram_tensor("mhi", [NT * 128, JT * 512], f8e4,
                           kind="ExternalInput")
    # -sq pre-broadcast to 128 partitions (fp32 exact): the drain adds it
    nsq_d = nc.dram_tensor("nsq", [128, N], f32, kind="ExternalInput")
    # stationary side (this core's 512 rows)
    st16_d = nc.dram_tensor("st16", [128, JT * 512], f16,
                            kind="ExternalInput")
    sta_d = nc.dram_tensor("sta", [128, JT * 512], f8e4,
                           kind="ExternalInput")
    stb_d = nc.dram_tensor("stb", [128, JT * 512], f8e5,
                           kind="ExternalInput")
    out_d = nc.dram_tensor("out", [M, S], f32, kind="ExternalOutput")

    with tile.TileContext(nc) as tc:
        with (
            tc.tile_pool(name="weights", bufs=1) as wpool,
            tc.tile_pool(name="s", bufs=1) as spool,
            tc.tile_pool(name="m8", bufs=1) as m8pool,
            tc.tile_pool(name="fin", bufs=2) as fpool,
            tc.tile_pool(name="mask", bufs=1) as mpool,
            tc.tile_pool(name="outp", bufs=2) as opool,
            tc.tile_pool(name="psum", bufs=8, space="PSUM") as psum,
        ):
            # critical path first: the fp16 stationary + first moving chunk,
            # both split so round-0 matmuls can start before the full tile
            # lands (plane j is consumed in emission order)
            st16 = wpool.tile([128, JT, 512], f16, tag="st16", name="st16")
            m16_sb = [wpool.tile([128, JT, 512], f16, tag=f"m16_{n}",
                                 name=f"m16_{n}") for n in range(NT)]
            # finest splits first so the j=0 matmul starts earlier; all
            # on the Sync queue (ACT/GpSimd dma_start = slower SW-DGE path,
            # measured +5us)
            nc.sync.dma_start(out=st16[:, 0:1, :], in_=st16_d[:, :512])
            nc.sync.dma_start(out=m16_sb[0][:, 0:2, :],
                              in_=m16_d[0:128, :2 * 512])
            nc.sync.dma_start(out=st16[:, 1:3, :],
                              in_=st16_d[:, 512:3 * 512])
            nc.sync.dma_start(out=m16_sb[0][:, 2:4, :],
                              in_=m16_d[0:128, 2 * 512:4 * 512])
            nc.sync.dma_start(out=st16[:, 3:6, :], in_=st16_d[:, 3 * 512:])
            nc.sync.dma_start(out=m16_sb[0][:, 4:6, :],
                              in_=m16_d[0:128, 4 * 512:])
            sta = wpool.tile([128, JT, 512], f8e4, tag="sta", name="sta")
            nc.sync.dma_start(out=sta, in_=sta_d[:, :])
            stb = wpool.tile([128, JT, 512], f8e5, tag="stb", name="stb")
            nc.sync.dma_start(out=stb, in_=stb_d[:, :])
            mlo_sb = [wpool.tile([128, JT, 512], f8e5, tag=f"mlo_{n}",
                                 name=f"mlo_{n}") for n in range(NT)]
            mhi_sb = [wpool.tile([128, JT, 512], f8e4, tag=f"mhi_{n}",
                                 name=f"mhi_{n}") for n in range(NT)]
            nc.sync.dma_start(out=mlo_sb[0], in_=mlo_d[0:128, :])
            nc.sync.dma_start(out=mhi_sb[0], in_=mhi_d[0:128, :])
            nsq_sb = [wpool.tile([128, 512], f32, tag=f"nsq_{n}",
                                 name=f"nsq_{n}") for n in range(NT)]
            nc.sync.dma_start(out=nsq_sb[0], in_=nsq_d[:, 0:512])
            for n in range(1, NT):
                nc.sync.dma_start(out=m16_sb[n],
                                  in_=m16_d[n * 128:(n + 1) * 128, :])
                nc.sync.dma_start(out=mlo_sb[n],
                                  in_=mlo_d[n * 128:(n + 1) * 128, :])
                nc.sync.dma_start(out=mhi_sb[n],
                                  in_=mhi_d[n * 128:(n + 1) * 128, :])
                nc.sync.dma_start(out=nsq_sb[n],
                                  in_=nsq_d[:, n * 512:(n + 1) * 512])

            s_sb = [spool.tile([128, N], f32, tag=f"s{rt}", name=f"s{rt}")
                    for rt in range(RT)]
            m8s = [m8pool.tile([128, 64], f32, tag=f"m8_{rt}",
                               name=f"m8_{rt}") for rt in range(RT)]

            def finalize(rt):
                # sigma = 16th largest of the union of the 8 block top-8s
                c8 = fpool.tile([128, 8], f32, tag="c8", name="c8")
                m8x = fpool.tile([128, 64], f32, tag="m8x", name="m8x")
                d8 = fpool.tile([128, 8], f32, tag="d8", name="d8")
                nc.vector.max(out=c8, in_=m8s[rt])
                nc.vector.match_replace(out=m8x, in_to_replace=c8,
                                        in_values=m8s[rt], imm_value=NEG)
                nc.vector.max(out=d8, in_=m8x)
                sigma = d8[:, 7:8]

                # neighbor mask (s >= sigma): two single-ALU-pass is_ge
                # halves + one bf16 add beat one fused two-op pass
                eng = nc.vector
                H = N // 2
                mask = mpool.tile([128, H], bf16, tag="maskA",
                                  name=f"maskA{rt}")
                maskB = mpool.tile([128, H], bf16, tag="maskB",
                                   name=f"maskB{rt}")
                eng.tensor_scalar(mask, s_sb[rt][:, :H], sigma, None,
                                  op0=mybir.AluOpType.is_ge)
                eng.tensor_scalar(maskB, s_sb[rt][:, H:], sigma, None,
                                  op0=mybir.AluOpType.is_ge)
                eng.tensor_add(mask, mask, maskB)
                w = H // 2
                while w > S:
                    eng.tensor_add(mask[:, :w], mask[:, :w],
                                   mask[:, w:2 * w])
                    w //= 2
                o = opool.tile([128, S], f32, tag=f"o{rt % 2}",
                               name=f"o{rt}")
                eng.tensor_add(o, mask[:, :S], mask[:, S:2 * S])
                nc.sync.dma_start(
                    out=out_d[rt * 128:(rt + 1) * 128, :], in_=o)

            def mm16(ps, n, rt):
                rsl = slice(rt * 128, (rt + 1) * 128)
                for j in range(JT):
                    nc.tensor.matmul(
                        ps, st16[:, j:j + 1, rsl],
                        m16_sb[n][:, j:j + 1, :],
                        start=(j == 0), stop=False)

            def mmdr(ps, n, rt):
                rsl = slice(rt * 128, (rt + 1) * 128)
                for c in range(JT // 2):
                    nc.tensor.matmul(
                        ps, sta[:, 2 * c:2 * c + 2, rsl],
                        mlo_sb[n][:, 2 * c:2 * c + 2, :],
                        start=False, stop=False, perf_mode=DR)
                for c in range(JT // 2):
                    nc.tensor.matmul(
                        ps, stb[:, 2 * c:2 * c + 2, rsl],
                        mhi_sb[n][:, 2 * c:2 * c + 2, :],
                        start=False, stop=(c == JT // 2 - 1), perf_mode=DR)

            def drain(ps, n, rt):
                # drain adds the exact fp32 -sq row; block top-8 follows
                nsl = slice(n * 512, (n + 1) * 512)
                nc.vector.tensor_add(s_sb[rt][:, nsl], ps, nsq_sb[n])
                nc.vector.max(out=m8s[rt][:, n * 8:(n + 1) * 8],
                              in_=s_sb[rt][:, nsl])

            def block(n, rt):
                ps = psum.tile([128, 512], f32, tag="ps", name="ps")
                mm16(ps, n, rt)
                mmdr(ps, n, rt)
                drain(ps, n, rt)

            # round 0: all fp16 MMs first (their operands land first),
            # fp8 DR MMs after - by then sta/stb/mlo0/mhi0 have arrived,
            # so the PE does not stall waiting on the fp8 tiles
            ps0 = [psum.tile([128, 512], f32, tag="ps", name=f"ps0_{rt}")
                   for rt in range(RT)]
            for rt in range(RT):
                mm16(ps0[rt], 0, rt)
            for rt in range(RT):
                mmdr(ps0[rt], 0, rt)
                drain(ps0[rt], 0, rt)

            # phase A: chunk-outer over the first chunks (each moving
            # chunk is reused by all 4 row-tiles right after it lands)
            NA = 3
            for n in range(1, NA):
                for rt in range(RT):
                    block(n, rt)
            # phase B: row-outer so each row-tile finishes all its chunks
            # early and its finalize overlaps the next row-tile's matmuls
            for rt in range(RT):
                for n in range(NA, NT):
                    block(n, rt)
                finalize(rt)

    nc.compile()
    return nc


def _prep_inputs(x):
    import ml_dtypes
    e4 = ml_dtypes.float8_e4m3
    e5 = ml_dtypes.float8_e5m2

    flat = np.asarray(x, dtype=np.float32).reshape(N, D)
    sq = (flat.astype(np.float64) ** 2).sum(1)

    hi = flat.astype(np.float16)                      # rhs fp16
    lo = flat - hi.astype(np.float32)                 # rhs residual
    hi2 = (2.0 * flat).astype(np.float16)             # lhs fp16
    lo2 = 2.0 * flat - hi2.astype(np.float32)         # lhs residual

    def planes(a):
        # [4096, 768] -> [128, 6, 4096]: plane j row p = feature j*128+p
        return np.ascontiguousarray(
            a.T.reshape(JT, 128, N).transpose(1, 0, 2))

    def chunks(a):
        # [128, 6, 4096] -> [1024, 3072]: chunk-major moving layout
        return np.ascontiguousarray(
            a.reshape(128, JT, NT, 512).transpose(2, 0, 1, 3)
            .reshape(NT * 128, JT * 512))

    m16 = chunks(planes(hi))                          # fp16
    mlo = chunks(planes(lo.astype(e5).astype(np.float32))).astype(e5)
    mhi = chunks(planes(flat.astype(e4).astype(np.float32))).astype(e4)
    nsq = np.ascontiguousarray(
        np.broadcast_to((-sq).astype(np.float32), (128, N)))

    st16_full = planes(hi2)                           # [128, 6, 4096] fp16
    sta_full = planes((2.0 * flat).astype(e4).astype(np.float32))
    stb_full = planes(lo2.astype(e5).astype(np.float32))

    def st_core(a, c, dt):
        return np.ascontiguousarray(
            a[:, :, c * M:(c + 1) * M].reshape(128, JT * 512)).astype(dt)

    return m16, mlo, mhi, nsq, st16_full, sta_full, stb_full, st_core


def kernel(x, k):
    assert int(k) == 16
    import ml_dtypes
    e4 = ml_dtypes.float8_e4m3
    e5 = ml_dtypes.float8_e5m2
    (m16, mlo, mhi, nsq,
     st16_full, sta_full, stb_full, st_core) = _prep_inputs(x)

    if "nc" not in _cache:
        _cache["nc"] = _build()
    nc = _cache["nc"]

    in_maps = [
        {"m16": m16, "mlo": mlo, "mhi": mhi, "nsq": nsq,
         "st16": st_core(st16_full, c, np.float16),
         "sta": st_core(sta_full, c, e4),
         "stb": st_core(stb_full, c, e5)}
        for c in range(NCORES)
    ]

    from concourse.bass_utils import run_bass_kernel_spmd
    trace = bool(os.environ.get("KNN_TRACE"))
    if trace:
        try:
            from antenv.axon_hooks import get_axon_ntff_profile_hook
            if get_axon_ntff_profile_hook() is None:
                trace = False
        except ImportError:
            trace = False
    res = run_bass_kernel_spmd(nc, in_maps, core_ids=list(range(NCORES)),
                               trace=trace)
    if trace and res.exec_time_ns is not None:
        print(f"HW exec time: {res.exec_time_ns} ns")
        _cache["exec_time_ns"] = res.exec_time_ns

    out = np.concatenate([r["out"] for r in res.results], axis=0)
    return out.reshape(B, S, S)
